# revision 1
# baseline (speedup 1.0000x reference)
"""MinibatchDiscrimination Trainium2 kernel v2 (8-core SPMD, full I/O).

Math (reference):
  act = einsum('bd,kdm->bkm', x, W)        # (512, 64, 16)
  l1[i,j,k] = sum_m |act[i,k,m] - act[j,k,m]|
  feats[i,k] = sum_j exp(-l1[i,j,k]) + b[k]
  out = concat([x, feats], axis=1)         # (512, 320)

v2 strategy (~2x less pairwise work than v1 via L1 symmetry):
  Each unordered pair {i, j} is evaluated once. Row i's core computes a
  258-wide circular column window j in [i, i+257]; the exp of each window
  element lands in BOTH a row-sum (accum_out -> feats[i]) and a column
  accumulator (identity-matmul into PSUM -> partial feats[j]) which the
  host reduces across cores. Distance-256 pairs are deduplicated via a
  per-core mask (+100 into l1 pre-exp, killing the exp) delivered as the
  rank-1 matmul that opens each PSUM accumulation group; the diagonal is
  counted twice and the host subtracts 1.

Per-core pipeline, pair q = local rows (2q, 2q+1), window cols [2q, 2q+257]
of the 336-wide per-core-rotated activation tensor actTx:
  DVE:  12 of 16 tile-subtracts (actTx window - act_row col) -> fp8e4
        (fast 4x output mode) + one sign-clear AND per row over the u16
        view of the packed fp8 buffer (= abs)
  ACT:  4 of 16 tiles via fused Abs(x + bias) -> fp8; per row exp(-l1)
        whose accum_out yields the row-sum feats contribution
  PE:   actTx prologue (bf16); per row: rank-1 mask matmul (group open),
        4 DoubleRow fp8 matmuls (2 km-tiles each, 2x rate) accumulating
        the masked l1, identity col-accumulate of exp into acc2 PSUM
  Host: scatter-add rotated column partials, subtract diag dup.
"""

import sys

sys.path.insert(0, "/opt/trn_rl_repo")

import numpy as np
import ml_dtypes

import concourse.bass as bass
import concourse.bacc as bacc
import concourse.tile as tile
from concourse import mybir
from concourse import bass_utils

B, D, K, M = 512, 256, 64, 16
KM = K * M          # 1024
NT = KM // 128      # 8 km-tiles
NCORES = 8
RPC = B // NCORES   # 64 rows per core
NPAIR = RPC // 2    # 32 pairs per core
W = 260             # window width (multiple of 4 for DVE 4x mode)
WP = 264            # padded inner stride (4B-aligned slices)
EXT = 336           # extended actT columns (>= 62 + 258, padded)
ACC = 322           # column accumulator width (62 + 258, even)
NACT = 3            # km-tiles per row handled by ACT (of NT=8)
MASKV = 100.0       # l1 offset for masked window columns

FP32 = mybir.dt.float32
BF16 = mybir.dt.bfloat16
FP8 = mybir.dt.float8e4
U16 = mybir.dt.uint16


def build_bass():
    nc = bacc.Bacc(None, target_bir_lowering=False, debug=False)

    xTr = nc.declare_dram_parameter("xTr", [D, EXT], BF16, isOutput=False)
    w2 = nc.declare_dram_parameter("w2", [D, KM], BF16, isOutput=False)
    xi = nc.declare_dram_parameter("xi", [RPC, D], FP32, isOutput=False)
    brep = nc.declare_dram_parameter("brep", [NPAIR, 2 * K], FP32, isOutput=False)
    gdr = nc.declare_dram_parameter("gdr", [128, NT // 2, 2, K], FP8, isOutput=False)
    identB = nc.declare_dram_parameter("identB", [64, 64], BF16, isOutput=False)
    identF = nc.declare_dram_parameter("identF", [128, 128], FP32, isOutput=False)
    ones1 = nc.declare_dram_parameter("ones1", [1, K], BF16, isOutput=False)
    msrc = nc.declare_dram_parameter("msrc", [2, W], BF16, isOutput=False)
    out = nc.declare_dram_parameter("out", [RPC, D + K], FP32, isOutput=True)
    outc = nc.declare_dram_parameter("outc", [K, ACC], FP32, isOutput=True)

    with tile.TileContext(nc) as tc:
        with (
            tc.tile_pool(name="consts", bufs=1) as consts,
            tc.tile_pool(name="work", bufs=4) as work,
            tc.tile_pool(name="small", bufs=4) as small,
            tc.tile_pool(name="psum_a", bufs=2, space="PSUM") as psum_a,
            tc.tile_pool(name="psum_l", bufs=4, space="PSUM") as psum_l,
            tc.tile_pool(name="psum_c", bufs=1, space="PSUM") as psum_c,
        ):
            # ---- load inputs ----
            xTr_b = consts.tile([128, 2, EXT], BF16, tag="xTr_b")
            nc.sync.dma_start(out=xTr_b, in_=xTr[:].rearrange("(h p) b -> p h b", p=128))
            w2_b = consts.tile([128, 2, KM], BF16, tag="w2_b")
            nc.sync.dma_start(out=w2_b, in_=w2[:].rearrange("(h p) n -> p h n", p=128))
            xi_f = consts.tile([RPC, D], FP32, tag="xi_f")
            nc.sync.dma_start(out=xi_f, in_=xi[:])
            brep_f = consts.tile([NPAIR, 2 * K], FP32, tag="brep_f")
            nc.sync.dma_start(out=brep_f, in_=brep[:])
            gdr_b = consts.tile([128, NT // 2, 2, K], FP8, tag="gdr_b")
            nc.sync.dma_start(out=gdr_b, in_=gdr[:])
            idB = consts.tile([64, 64], BF16, tag="idB")
            nc.sync.dma_start(out=idB, in_=identB[:])
            idF = consts.tile([128, 128], FP32, tag="idF")
            nc.sync.dma_start(out=idF, in_=identF[:])
            ones_b = consts.tile([1, K], BF16, tag="ones_b")
            nc.sync.dma_start(out=ones_b, in_=ones1[:])
            msrc0_b = consts.tile([1, W], BF16, tag="msrc0_b")
            nc.sync.dma_start(out=msrc0_b, in_=msrc[0:1, :])
            msrc1_b = consts.tile([1, W], BF16, tag="msrc1_b")
            nc.sync.dma_start(out=msrc1_b, in_=msrc[1:2, :])

            # ---- prologue: actTx [128, NT, EXT] bf16; negIT [128, NT, RPC] f32 ----
            actTx = consts.tile([128, NT, EXT], BF16, tag="actTx")
            negIT = consts.tile([128, NT, RPC], FP32, tag="negIT")
            for t in range(NT):
                pa = psum_a.tile([128, EXT], FP32, tag="pa")
                for dh in range(2):
                    nc.tensor.matmul(
                        pa,
                        w2_b[:, dh, t * 128:(t + 1) * 128],
                        xTr_b[:, dh, :],
                        start=(dh == 0),
                        stop=(dh == 1),
                    )
                if t % 2 == 0:
                    nc.scalar.copy(actTx[:, t, :], pa)
                else:
                    nc.vector.tensor_scalar(
                        out=actTx[:, t, :], in0=pa, scalar1=0.0, scalar2=None,
                        op0=mybir.AluOpType.add,
                    )
                # own rows live at local cols 0..RPC; negate their bf16 values
                nc.vector.tensor_scalar(
                    out=negIT[:, t, :], in0=actTx[:, t, 0:RPC],
                    scalar1=-1.0, scalar2=None, op0=mybir.AluOpType.mult,
                )

            # zero tile for the col-accumulator group opener
            zt = consts.tile([64, ACC], BF16, tag="zt")
            nc.vector.memset(zt, 0.0)

            # ---- column accumulator PSUM [64, ACC] ----
            acc2 = psum_c.tile([64, ACC], FP32, tag="acc2")
            nc.tensor.matmul(acc2, idB, zt, start=True, stop=False,
                             skip_group_check=True)

            # ---- main pair loop ----
            featsP0 = consts.tile([K, NPAIR], FP32, tag="featsP0")
            featsP1 = consts.tile([K, NPAIR], FP32, tag="featsP1")
            featsP = [featsP0, featsP1]
            pending = []
            for q in range(NPAIR):
                lo = 2 * q
                adif0 = work.tile([128, NT // 2, 2, WP], FP8, tag="adif0")
                adif1 = work.tile([128, NT // 2, 2, WP], FP8, tag="adif1")
                adif = [adif0, adif1]
                for e in range(2):
                    il = lo + e
                    for t in range(NT):
                        dst = adif[e][:, t // 2, t % 2, 0:W]
                        src = actTx[:, t, lo:lo + W]
                        if t >= NT - NACT:
                            nc.scalar.activation(
                                out=dst, in_=src,
                                func=mybir.ActivationFunctionType.Abs,
                                bias=negIT[:, t, il:il + 1], scale=1.0,
                            )
                        else:
                            nc.vector.tensor_scalar(
                                out=dst, in0=src,
                                scalar1=negIT[:, t, il:il + 1], scalar2=None,
                                op0=mybir.AluOpType.add,
                            )
                    au = adif[e].bitcast(U16)
                    nc.vector.tensor_scalar(
                        out=au[:, 0:2, :, :], in0=au[:, 0:2, :, :],
                        scalar1=0x7F7F, scalar2=None,
                        op0=mybir.AluOpType.bitwise_and,
                    )
                    nc.vector.tensor_scalar(
                        out=au[:, 2, 0, :], in0=au[:, 2, 0, :],
                        scalar1=0x7F7F, scalar2=None,
                        op0=mybir.AluOpType.bitwise_and,
                    )

                for scr_p, lo_p in pending:
                    nc.tensor.matmul(
                        acc2[:, lo_p:lo_p + W], idB, scr_p,
                        start=False, stop=False, skip_group_check=True,
                    )
                pending = []
                for e in range(2):
                    pl = psum_l.tile([K, W], FP32, tag="pl")
                    nc.tensor.matmul(pl, ones_b,
                                     msrc0_b if e == 0 else msrc1_b,
                                     start=True, stop=False)
                    for tp in range(NT // 2):
                        nc.tensor.matmul(
                            pl,
                            gdr_b[:, tp, :, :],
                            adif[e][:, tp, :, 0:W],
                            start=False,
                            stop=(tp == NT // 2 - 1),
                            perf_mode=mybir.MatmulPerfMode.DoubleRow,
                        )
                    scr = small.tile([K, W], BF16, tag=f"scr{e}")
                    nc.scalar.activation(
                        out=scr, in_=pl,
                        func=mybir.ActivationFunctionType.Exp,
                        scale=-1.0,
                        accum_out=featsP[e][:, q:q + 1],
                    )
                    pending.append((scr, lo))
            for i, (scr_p, lo_p) in enumerate(pending):
                nc.tensor.matmul(
                    acc2[:, lo_p:lo_p + W], idB, scr_p,
                    start=False, stop=(i == len(pending) - 1),
                    skip_group_check=True,
                )

            # ---- outputs ----
            accF = consts.tile([K, ACC], FP32, tag="accF")
            nc.scalar.copy(accF, acc2)
            nc.sync.dma_start(out=outc[:], in_=accF)

            outf = consts.tile([NPAIR, 2 * K], FP32, tag="outf")
            for e in range(2):
                ptr = psum_c.tile([NPAIR, K], FP32, tag="ptr")
                nc.tensor.transpose(ptr, featsP[e], idF[0:K, 0:K])
                nc.vector.tensor_tensor(
                    out=outf[:, e * K:(e + 1) * K], in0=ptr,
                    in1=brep_f[:, e * K:(e + 1) * K], op=mybir.AluOpType.add,
                )
            nc.sync.dma_start(
                out=out[:, D:D + K].rearrange("(c e) k -> c e k", e=2),
                in_=outf.rearrange("c (e k) -> c e k", e=2),
            )
            nc.sync.dma_start(out=out[:, 0:D], in_=xi_f)

    nc.compile()
    return nc


_NC_CACHE = None


def _get_nc():
    global _NC_CACHE
    if _NC_CACHE is None:
        _NC_CACHE = build_bass()
    return _NC_CACHE


def make_in_maps(x, W_, b):
    x = np.asarray(x, dtype=np.float32)
    W_ = np.asarray(W_, dtype=np.float32)
    b = np.asarray(b, dtype=np.float32)
    xT = np.ascontiguousarray(x.T)                       # (256, 512) fp32
    w2 = np.ascontiguousarray(
        W_.transpose(1, 0, 2).reshape(D, KM)).astype(ml_dtypes.bfloat16)
    brep = np.ascontiguousarray(
        np.broadcast_to(np.tile(b, 2)[None, :], (NPAIR, 2 * K)))

    # gdr[p, tp, q, k] = 1 iff k == 8*(2*tp+q) + p//16
    p = np.arange(128)[:, None, None, None]
    tp = np.arange(NT // 2)[None, :, None, None]
    qq = np.arange(2)[None, None, :, None]
    k = np.arange(K)[None, None, None, :]
    gdr = (k == NT * (2 * tp + qq) + p // M).astype(ml_dtypes.float8_e4m3)

    identB = np.eye(64, dtype=ml_dtypes.bfloat16)
    identF = np.eye(128, dtype=np.float32)
    ones1 = np.ones((1, K), dtype=ml_dtypes.bfloat16)

    in_maps = []
    for c in range(NCORES):
        rows = slice(c * RPC, (c + 1) * RPC)
        cols = (c * RPC + np.arange(EXT)) % B
        xTr = np.ascontiguousarray(xT[:, cols]).astype(ml_dtypes.bfloat16)
        msrc = np.zeros((2, W), dtype=ml_dtypes.bfloat16)
        keep256 = c < NCORES // 2
        msrc[0, 256] = 0.0 if keep256 else MASKV
        msrc[0, 257:260] = MASKV
        msrc[1, 0] = MASKV
        msrc[1, 257] = 0.0 if keep256 else MASKV
        msrc[1, 258:260] = MASKV
        in_maps.append({
            "xTr": xTr,
            "w2": w2,
            "xi": np.ascontiguousarray(x[rows]),
            "brep": brep,
            "gdr": gdr,
            "identB": identB,
            "identF": identF,
            "ones1": ones1,
            "msrc": msrc,
        })
    return in_maps


def kernel(x, W, b, _trace=False, _tmpdir=None):
    nc = _get_nc()
    in_maps = make_in_maps(x, W, b)
    res = bass_utils.run_bass_kernel_spmd(
        nc, in_maps, core_ids=list(range(NCORES)),
        trace=_trace, tmpdir=_tmpdir,
    )
    out = np.concatenate(
        [res.results[c]["out"] for c in range(NCORES)], axis=0)
    # host-side reduce of the column partials
    feats_add = np.zeros((B, K), dtype=np.float32)
    for c in range(NCORES):
        cadd = res.results[c]["outc"]                # [K, ACC]
        idx = (c * RPC + np.arange(ACC)) % B
        np.add.at(feats_add, idx, cadd.T)
    out[:, D:D + K] += feats_add - 1.0
    if _trace:
        return out, res
    return out

